# revision 7
# baseline (speedup 1.0000x reference)
"""Trainium2 Bass kernel for nn_CPAMDec_Mix (dual cross-attention decoder block).

Math per batch sample b (C=512, C4=128, K=64, N=W*H=4096):
    pv1 = scale  * (wv @ y1^T + bv)   [C, K]   (scale folded in at setup)
    pv2 = scale1 * (wv @ y2^T + bv)   [C, K]
    q^T = wq @ x2 + bq                [C4, N]
    kk  = y2 @ wk^T + bk              [K, C4]
    e^T = kk @ q^T                    [K, N]   (energy, transposed layout)
    attT = exp(|e^T|) / colsum        [K, N]   (softmax over k = partition dim)
    out1 = pv1 @ attT + x1
    out2 = pv2 @ attT + x2

Sharding: pure data parallel - sample b on core b (B == n_cores == 8).

The attention stays in [K, N] layout end to end: the energy is produced
directly transposed (one matmul per 512-wide tile), the softmax column
sums are broadcast to all K partitions with an all-ones matmul, and the
output GEMMs consume attT without any transposes. Residual adds are
split across the vector (out1) and gpsimd (out2) engines. x1 and both
outputs travel as bf16 (error ~4e-3 l2, well under the 1e-2 gate),
cutting HBM traffic from 33.7 MB to 21.7 MB per core. All loads are
front-loaded on the sync HW-DGE queue; stores go on the scalar queue,
deferred one pipeline phase so they never stall the attention chain.
"""

import numpy as np

import concourse.bass as bass
import concourse.mybir as mybir
import concourse.tile as tile
from concourse import bacc
from concourse.bass_utils import run_bass_kernel_spmd

F32 = mybir.dt.float32
F32R = mybir.dt.float32r
BF16 = mybir.dt.bfloat16
U32 = mybir.dt.uint32
OP = mybir.AluOpType
AF = mybir.ActivationFunctionType

B, C, W, H, K = 8, 512, 64, 64, 64
C4 = C // 4
N = W * H            # 4096
NT = 512             # matmul tile (free-dim columns)
NQ = 1024            # quarter width (epilogue/store chunk)
NHALF = 2048         # x1 load chunk
CC = C // 128        # 4 chunks of 128 over the channel dim

_CACHE = {}


def _load_chunked(nc, dst_tile, src_dram, inner, lo=0):
    """One DMA: [CC*128, inner] DRAM slab -> [128, CC*inner] SBUF tile
    (row chunk cc lands at columns cc*inner..)."""
    nc.sync.dma_start(
        out=dst_tile[:].rearrange("p (c n) -> p c n", c=CC),
        in_=src_dram[:].rearrange("(c p) n -> p c n", p=128)[:, :, lo : lo + inner],
    )


def _build_nc():
    nc = bacc.Bacc("TRN2", target_bir_lowering=False, debug=False)

    x1_d = nc.dram_tensor("x1", [C, N], BF16, kind="ExternalInput")
    x2_d = nc.dram_tensor("x2", [C, N], F32R, kind="ExternalInput")
    y1T_d = nc.dram_tensor("y1T", [C, K], F32, kind="ExternalInput")
    y2T_d = nc.dram_tensor("y2T", [C, K], F32, kind="ExternalInput")
    wqT_d = nc.dram_tensor("wqT", [C, C4], F32R, kind="ExternalInput")
    wkT_d = nc.dram_tensor("wkT", [C, C4], F32, kind="ExternalInput")
    wvT_d = nc.dram_tensor("wvT", [C, C], F32, kind="ExternalInput")
    # packed per-partition vectors: [bq | bk | scale | scale1]
    vecs_d = nc.dram_tensor("vecs", [C4, 4], F32, kind="ExternalInput")
    # packed rows: [bv (512) | ones (64)]
    rows_d = nc.dram_tensor("rows", [1, C + K], F32, kind="ExternalInput")
    ones_d = nc.dram_tensor("ones64", [K, K], F32R, kind="ExternalInput")
    out1_d = nc.dram_tensor("out1", [C, N], BF16, kind="ExternalOutput")
    out2_d = nc.dram_tensor("out2", [C, N], BF16, kind="ExternalOutput")

    with tile.TileContext(nc) as tc:
        with (
            tc.tile_pool(name="const", bufs=1) as const,
            tc.tile_pool(name="qpool", bufs=2) as qpool,
            tc.tile_pool(name="spool", bufs=2) as spool,
            tc.tile_pool(name="apool", bufs=4) as apool,
            tc.tile_pool(name="o1pool", bufs=5) as o1pool,
            tc.tile_pool(name="o2pool", bufs=5) as o2pool,
            tc.tile_pool(name="tpool", bufs=3) as tpool,
            tc.tile_pool(name="psq", bufs=2, space="PSUM") as psq,
            tc.tile_pool(name="pe", bufs=2, space="PSUM") as pe,
            tc.tile_pool(name="pso", bufs=4, space="PSUM") as pso,
        ):
            # ---- loads, in the order the pipeline needs them ----
            wqT_sb = const.tile([128, CC * C4], F32R)
            _load_chunked(nc, wqT_sb, wqT_d, C4)
            wkT_sb = const.tile([128, CC * C4], F32)
            _load_chunked(nc, wkT_sb, wkT_d, C4)
            y2T_sb = const.tile([128, CC * K], F32)
            _load_chunked(nc, y2T_sb, y2T_d, K)
            vecs_sb = const.tile([C4, 4], F32)
            nc.sync.dma_start(out=vecs_sb[:], in_=vecs_d[:])
            rows_sb = const.tile([1, C + K], F32)
            nc.sync.dma_start(out=rows_sb[:], in_=rows_d[:])
            bq_sb = vecs_sb[:, 0:1]
            bk_sb = vecs_sb[:, 1:2]
            sc1_sb = vecs_sb[0:K, 2:3]
            sc2_sb = vecs_sb[0:K, 3:4]
            bv_sb = rows_sb[:, 0:C]
            onesrow_sb = rows_sb[:, C : C + K]

            x2_sb = []
            t = const.tile([128, CC * NQ], F32R, tag="x2_0")
            _load_chunked(nc, t, x2_d, NQ, lo=0)
            x2_sb.append(t)
            wvT_sb = const.tile([128, CC * C], F32)
            _load_chunked(nc, wvT_sb, wvT_d, C)
            y1T_sb = const.tile([128, CC * K], F32)
            _load_chunked(nc, y1T_sb, y1T_d, K)
            for q in range(1, N // NQ):
                t = const.tile([128, CC * NQ], F32R, tag=f"x2_{q}")
                _load_chunked(nc, t, x2_d, NQ, lo=q * NQ)
                x2_sb.append(t)
            x1_sb = []
            for h in range(N // NHALF):
                t = const.tile([128, CC * NHALF], BF16, tag=f"x1_{h}")
                _load_chunked(nc, t, x1_d, NHALF, lo=h * NHALF)
                x1_sb.append(t)

            # ---- all-ones [K, K] for the softmax column-sum broadcast ----
            ones64 = const.tile([K, K], F32R)
            nc.sync.dma_start(out=ones64[:], in_=ones_d[:])

            # ---- kk^T [C4, K] (lhsT of every energy matmul) ----
            pkk = psq.tile([128, NT], F32, tag="psq")
            for cc in range(CC):
                nc.tensor.matmul(
                    pkk[:, 0:K],
                    lhsT=wkT_sb[:, cc * C4 : (cc + 1) * C4],
                    rhs=y2T_sb[:, cc * K : (cc + 1) * K],
                    start=(cc == 0),
                    stop=(cc == CC - 1),
                )
            kkT_sb = const.tile([C4, K], F32R)
            nc.scalar.activation(kkT_sb[:], pkk[:, 0:K], AF.Identity, bias=bk_sb)

            # ---- pv^T [K, C] = scale * (y^T.T @ wvT + ones^T bv) ----
            pv_sb = []
            for yT_sb, sc in ((y1T_sb, sc1_sb), (y2T_sb, sc2_sb)):
                ppv = pe.tile([K, C], F32, tag="pe")
                for cc in range(CC):
                    nc.tensor.matmul(
                        ppv[:],
                        lhsT=yT_sb[:, cc * K : (cc + 1) * K],
                        rhs=wvT_sb[:, cc * C : (cc + 1) * C],
                        start=(cc == 0),
                        stop=False,
                    )
                nc.tensor.matmul(
                    ppv[:], lhsT=onesrow_sb, rhs=bv_sb, start=False, stop=True
                )
                pv = const.tile([K, C], F32R, tag=f"pv_{len(pv_sb)}")
                nc.scalar.activation(pv[:], ppv[:], AF.Copy, scale=sc)
                pv_sb.append(pv)
            pv1T_sb, pv2T_sb = pv_sb

            def attention(q):
                """Two 512-wide tiles of attT [K, NT] for quarter q."""
                aTs = []
                for half in range(NQ // NT):
                    lo = half * NT
                    psq_t = psq.tile([128, NT], F32, tag="psq")
                    for cc in range(CC):
                        nc.tensor.matmul(
                            psq_t[:],
                            lhsT=wqT_sb[:, cc * C4 : (cc + 1) * C4],
                            rhs=x2_sb[q][:, cc * NQ + lo : cc * NQ + lo + NT],
                            start=(cc == 0),
                            stop=(cc == CC - 1),
                        )
                    qT = qpool.tile([C4, NT], F32R, tag="qT")
                    nc.scalar.activation(qT[:], psq_t[:], AF.Identity, bias=bq_sb)
                    eT = pe.tile([K, NT], F32, tag="pe")
                    nc.tensor.matmul(eT[:], lhsT=kkT_sb[:], rhs=qT[:])
                    eabs = spool.tile([K, NT], F32, tag="eabs")
                    nc.vector.tensor_scalar(
                        eabs[:].bitcast(U32),
                        eT[:].bitcast(U32),
                        0x7FFFFFFF,
                        None,
                        op0=OP.bitwise_and,
                    )
                    eexpT = spool.tile([K, NT], F32R, tag="eexpT")
                    nc.scalar.activation(eexpT[:], eabs[:], AF.Exp)
                    bsum = pe.tile([K, NT], F32, tag="pe")
                    nc.tensor.matmul(bsum[:], lhsT=ones64[:], rhs=eexpT[:])
                    rec = spool.tile([K, NT], F32, tag="rec")
                    nc.vector.reciprocal(rec[:], bsum[:])
                    aT = apool.tile([K, NT], F32R, tag="attT")
                    nc.vector.tensor_mul(aT[:], eexpT[:].bitcast(F32), rec[:])
                    aTs.append(aT)
                return aTs

            def outputs(q, aTs):
                """Output GEMMs + residual adds for quarter q; returns the
                o-tiles so their stores can be emitted one phase later."""
                otiles = []
                h = q // 2
                base = (q % 2) * NQ
                for cc in range(CC):
                    o1 = o1pool.tile([128, NQ], BF16, tag="o1")
                    o2 = o2pool.tile([128, NQ], BF16, tag="o2")
                    for half in range(NQ // NT):
                        lo = half * NT
                        po1 = pso.tile([128, NT], F32, tag="pso")
                        nc.tensor.matmul(
                            po1[:],
                            lhsT=pv1T_sb[:, cc * 128 : (cc + 1) * 128],
                            rhs=aTs[half][:],
                        )
                        po2 = pso.tile([128, NT], F32, tag="pso")
                        nc.tensor.matmul(
                            po2[:],
                            lhsT=pv2T_sb[:, cc * 128 : (cc + 1) * 128],
                            rhs=aTs[half][:],
                        )
                        nc.vector.tensor_add(
                            o1[:, lo : lo + NT],
                            po1[:],
                            x1_sb[h][:, cc * NHALF + base + lo : cc * NHALF + base + lo + NT],
                        )
                        # gpsimd cannot read PSUM: scalar stages po2 into
                        # SBUF, gpsimd does the SBUF+SBUF residual add.
                        t2 = tpool.tile([128, NT], F32, tag="t2")
                        nc.scalar.copy(t2[:], po2[:])
                        nc.gpsimd.tensor_add(
                            o2[:, lo : lo + NT],
                            t2[:],
                            x2_sb[q][:, cc * NQ + lo : cc * NQ + lo + NT].bitcast(F32),
                        )
                    otiles.append((cc, o1, o2))
                return otiles

            def stores(q, otiles):
                for cc, o1, o2 in otiles:
                    nc.scalar.dma_start(
                        out=out1_d[cc * 128 : (cc + 1) * 128, q * NQ : (q + 1) * NQ],
                        in_=o1[:],
                    )
                    nc.scalar.dma_start(
                        out=out2_d[cc * 128 : (cc + 1) * 128, q * NQ : (q + 1) * NQ],
                        in_=o2[:],
                    )

            # ---- software pipeline over the 4 quarters ----
            nq = N // NQ
            atts = {0: attention(0), 1: attention(1)}
            pending = {}
            for q in range(nq):
                pending[q] = outputs(q, atts.pop(q))
                if q + 2 < nq:
                    atts[q + 2] = attention(q + 2)
                if q - 1 in pending:
                    stores(q - 1, pending.pop(q - 1))
            stores(nq - 1, pending.pop(nq - 1))
    nc.compile()
    return nc


def _get_nc():
    if "nc" not in _CACHE:
        _CACHE["nc"] = _build_nc()
    return _CACHE["nc"]


def kernel(x1, y1, x2, y2, wq, bq, wk, bk, wv, bv, scale, scale1, **run_kwargs):
    import ml_dtypes

    x1 = np.asarray(x1, np.float32)
    x2 = np.asarray(x2, np.float32)
    y1 = np.asarray(y1, np.float32)
    y2 = np.asarray(y2, np.float32)
    vecs = np.stack(
        [
            np.asarray(bq, np.float32).reshape(C4),
            np.asarray(bk, np.float32).reshape(C4),
            np.full(C4, np.asarray(scale).reshape(-1)[0], np.float32),
            np.full(C4, np.asarray(scale1).reshape(-1)[0], np.float32),
        ],
        axis=1,
    )
    rows = np.concatenate(
        [np.asarray(bv, np.float32).reshape(C), np.ones(K, np.float32)]
    ).reshape(1, C + K)
    shared = {
        "wqT": np.ascontiguousarray(np.asarray(wq, np.float32).T),
        "wkT": np.ascontiguousarray(np.asarray(wk, np.float32).T),
        "wvT": np.ascontiguousarray(np.asarray(wv, np.float32).T),
        "vecs": np.ascontiguousarray(vecs),
        "rows": rows,
        "ones64": np.ones((K, K), np.float32),
    }
    in_maps = []
    for b in range(B):
        in_maps.append(
            {
                "x1": np.ascontiguousarray(
                    x1[b].reshape(C, N).astype(ml_dtypes.bfloat16)
                ),
                "x2": np.ascontiguousarray(x2[b].reshape(C, N)),
                "y1T": np.ascontiguousarray(y1[b].T),
                "y2T": np.ascontiguousarray(y2[b].T),
                **shared,
            }
        )
    nc = _get_nc()
    res = run_bass_kernel_spmd(nc, in_maps, list(range(B)), **run_kwargs)
    _CACHE["last_results"] = res
    out1 = np.stack(
        [
            np.asarray(res.results[b]["out1"]).astype(np.float32).reshape(C, W, H)
            for b in range(B)
        ]
    )
    out2 = np.stack(
        [
            np.asarray(res.results[b]["out2"]).astype(np.float32).reshape(C, W, H)
            for b in range(B)
        ]
    )
    return (out1, out2)
